# revision 1
# baseline (speedup 1.0000x reference)
"""Trainium2 Bass kernel for a 2-layer dual-direction gated GCN (DGGCN).

Contract: kernel(**inputs) takes the FULL unsharded inputs (as produced by
setup_inputs) and returns the FULL [N, D] float32 output.

Strategy (8 NeuronCores, node partition):
  - Nodes are remapped into a padded "slot" space: each core owns T_OWN tiles
    of 128 node slots. Self-loops are materialized as explicit (v, v) edges.
    Edges (incl. self-edges) are bucketed on the host by destination tile
    (forward) / source tile (reverse), and split per tile into lo/hi halves of
    the node space so indices fit dma_gather's int16 index format. Uniform
    B_LO/B_HI 128-edge block counts per tile make one SPMD program serve all
    cores.
  - Per layer: every core computes the dense transform h @ W for ALL nodes
    (layer 1 from the replicated x input; layer 2 from an AllGather of the
    gated layer-1 output), scales by rsqrt(deg) per direction and stores bf16
    message tables (two half-tables per direction) in local DRAM.
  - Aggregation per own dst tile: batched dma_gather of table rows for k
    tiles' edge blocks, one-hot selection matrices built on-chip from an iota
    constant (is_equal against per-edge local dst ids; padding uses an
    off-range sentinel), and PE matmuls accumulating segment sums in PSUM.
    Gates/elementwise run on ACT/DVE.
"""

import os
import sys

sys.path.insert(0, "/opt/trn_rl_repo")

import numpy as np

import concourse.bacc as bacc
import concourse.bass as bass
import concourse.tile as tile
from concourse import mybir
from concourse.bass_utils import run_bass_kernel_spmd
from concourse.masks import make_identity

F32 = mybir.dt.float32
BF16 = mybir.dt.bfloat16
I32 = mybir.dt.int32
I16 = mybir.dt.int16

W_CORES = 8
D = 128
PAD_DST = 200.0  # sentinel local-dst id (never matches iota 0..127)


# ---------------------------------------------------------------------------
# host-side graph preprocessing (index bucketing / sharding metadata only)
# ---------------------------------------------------------------------------


def _slot_of(v, sh_real, sh_pad):
    return (v // sh_real) * sh_pad + (v % sh_real)


def _pack16(flat):
    """Pack an int array [n] into dma_gather's [128, n//16] int16 layout:
    index i lives at partition i%16, column i//16, and the 16-partition block
    is replicated to all 8 GPSIMD-core stripes (partitions 16k..16k+15)."""
    n = flat.shape[0]
    assert n % 16 == 0
    return np.tile(flat.reshape(n // 16, 16).T, (8, 1))


def host_prepare(x, edge_index, n_real):
    w = W_CORES
    assert n_real % w == 0
    sh_real = n_real // w
    t_own = (sh_real + 127) // 128
    sh_pad = t_own * 128
    t_all = w * t_own
    np_pad = t_all * 128
    half = np_pad // 2
    assert half % 128 == 0 and half <= 32767

    src = np.asarray(edge_index[0], dtype=np.int64)
    dst = np.asarray(edge_index[1], dtype=np.int64)
    ss = _slot_of(src, sh_real, sh_pad)
    ds = _slot_of(dst, sh_real, sh_pad)
    selfs = _slot_of(np.arange(n_real, dtype=np.int64), sh_real, sh_pad)

    # degrees (counts only; rsqrt happens on device). +1 self-loop.
    deg_f = np.ones(np_pad, np.float32)
    deg_r = np.ones(np_pad, np.float32)
    np.add.at(deg_f, ds, 1.0)
    np.add.at(deg_r, ss, 1.0)

    # self-loops as explicit edges
    agg_f = np.concatenate([ds, selfs])
    gat_f = np.concatenate([ss, selfs])
    agg_r = np.concatenate([ss, selfs])
    gat_r = np.concatenate([ds, selfs])

    def bucket(agg_slot, gather_slot):
        tile_id = agg_slot // 128
        hi = (gather_slot >= half).astype(np.int64)
        order = np.lexsort((hi, tile_id))
        t_s = tile_id[order]
        g_s = gather_slot[order]
        h_s = hi[order]
        loc_s = (agg_slot[order] % 128).astype(np.float32)
        n_lo = np.bincount(t_s[h_s == 0], minlength=t_all)
        n_hi = np.bincount(t_s[h_s == 1], minlength=t_all)
        return t_s, g_s, h_s, loc_s, n_lo, n_hi

    bf = bucket(agg_f, gat_f)
    br = bucket(agg_r, gat_r)
    b_lo = int(max(bf[4].max(), br[4].max()) + 127) // 128
    b_hi = int(max(bf[5].max(), br[5].max()) + 127) // 128
    bt = b_lo + b_hi

    def build_tables(t_s, g_s, h_s, loc_s, n_lo, n_hi):
        # per-tile padded layouts: lo blocks then hi blocks
        idx_lo = np.zeros((t_all, b_lo * 128), np.int16)
        idx_hi = np.zeros((t_all, b_hi * 128), np.int16)
        dst_t = np.full((t_all, bt * 128), PAD_DST, np.float32)  # cast to bf16 per-core below
        starts_lo = np.zeros(t_all + 1, np.int64)
        np.cumsum(n_lo, out=starts_lo[1:])
        starts_hi = np.zeros(t_all + 1, np.int64)
        np.cumsum(n_hi, out=starts_hi[1:])
        # position of each edge within its (tile, half) group
        n_edges = len(t_s)
        grp_start = np.zeros(n_edges, np.int64)
        # edges sorted by (tile, half); compute position via running offsets
        tile_starts = np.zeros(t_all + 1, np.int64)
        np.cumsum(n_lo + n_hi, out=tile_starts[1:])
        pos_in_tile = np.arange(n_edges) - tile_starts[t_s]
        pos_lo = pos_in_tile  # valid where h_s == 0
        pos_hi = pos_in_tile - n_lo[t_s]  # valid where h_s == 1
        mlo = h_s == 0
        mhi = h_s == 1
        idx_lo[t_s[mlo], pos_lo[mlo]] = g_s[mlo].astype(np.int16)
        idx_hi[t_s[mhi], pos_hi[mhi]] = (g_s[mhi] - half).astype(np.int16)
        dst_t[t_s[mlo], pos_lo[mlo]] = loc_s[mlo]
        dst_t[t_s[mhi], b_lo * 128 + pos_hi[mhi]] = loc_s[mhi]
        return idx_lo, idx_hi, dst_t

    tbl_f = build_tables(*bf)
    tbl_r = build_tables(*br)

    # gather batch width (last batch may be smaller)
    k = min(4, t_own)
    nb = (t_own + k - 1) // k

    def per_core(idx_lo, idx_hi, dst_t):
        idxlo_l, idxhi_l, dst_l = [], [], []
        for c in range(w):
            ilo = idx_lo[c * t_own : (c + 1) * t_own]  # [t_own, b_lo*128]
            ihi = idx_hi[c * t_own : (c + 1) * t_own]
            # per batch: concat k tiles' lo (resp. hi) regions, pack int16
            plo = np.concatenate(
                [_pack16(ilo[g * k : min((g + 1) * k, t_own)].reshape(-1)) for g in range(nb)],
                axis=1,
            )
            phi = np.concatenate(
                [_pack16(ihi[g * k : min((g + 1) * k, t_own)].reshape(-1)) for g in range(nb)],
                axis=1,
            )
            dt = dst_t[c * t_own : (c + 1) * t_own]  # [t_own, bt*128]
            import ml_dtypes
            dT = np.ascontiguousarray(
                dt.reshape(t_own, bt, 128).transpose(2, 0, 1).reshape(128, t_own * bt)
            ).astype(ml_dtypes.bfloat16)
            idxlo_l.append(np.ascontiguousarray(plo))
            idxhi_l.append(np.ascontiguousarray(phi))
            dst_l.append(dT)
        return idxlo_l, idxhi_l, dst_l

    ilo_f, ihi_f, dstT_f = per_core(*tbl_f)
    ilo_r, ihi_r, dstT_r = per_core(*tbl_r)

    meta = dict(
        sh_real=sh_real, sh_pad=sh_pad, t_own=t_own, t_all=t_all, np_pad=np_pad,
        b_lo=b_lo, b_hi=b_hi, k=k, nb=nb,
    )
    tables = dict(
        ilo_f=ilo_f, ihi_f=ihi_f, dst_f=dstT_f,
        ilo_r=ilo_r, ihi_r=ihi_r, dst_r=dstT_r,
        deg_f=deg_f, deg_r=deg_r,
    )
    return meta, tables


# ---------------------------------------------------------------------------
# device program
# ---------------------------------------------------------------------------


def build_program(t_own, b_lo, b_hi, k, add_bc1=False, add_bc2=False, ag_chunks=2):
    w = W_CORES
    t_all = w * t_own
    np_pad = t_all * 128
    sh_pad = t_own * 128
    half_t = t_all // 2  # tiles per half-table
    bt = b_lo + b_hi
    nb = (t_own + k - 1) // k
    kg_of = [min(k, t_own - g * k) for g in range(nb)]
    lo_off = [sum(kg_of[:g]) * b_lo * 8 for g in range(nb + 1)]
    hi_off = [sum(kg_of[:g]) * b_hi * 8 for g in range(nb + 1)]
    lo_cols = lo_off[-1]  # packed idx columns per direction
    hi_cols = hi_off[-1]

    nc = bacc.Bacc(
        "TRN2", target_bir_lowering=False, debug=False, num_devices=w,
        num_swdge_queues=4,
    )

    # ---- external I/O -----------------------------------------------------
    xT_d = nc.dram_tensor("xT", [128, np_pad], F32, kind="ExternalInput")
    W1_d = nc.dram_tensor("W1", [128, 128], F32, kind="ExternalInput")
    W2_d = nc.dram_tensor("W2", [128, 128], F32, kind="ExternalInput")
    w11T_d = nc.dram_tensor("w11T", [128, 128], F32, kind="ExternalInput")
    w12T_d = nc.dram_tensor("w12T", [128, 128], F32, kind="ExternalInput")
    w21T_d = nc.dram_tensor("w21T", [128, 128], F32, kind="ExternalInput")
    w22T_d = nc.dram_tensor("w22T", [128, 128], F32, kind="ExternalInput")
    b1c_d = nc.dram_tensor("b1c", [128, 1], F32, kind="ExternalInput")
    b2c_d = nc.dram_tensor("b2c", [128, 1], F32, kind="ExternalInput")
    bc1r_d = nc.dram_tensor("bc1r", [128, 128], F32, kind="ExternalInput")
    bc2r_d = nc.dram_tensor("bc2r", [128, 128], F32, kind="ExternalInput")
    degfT_d = nc.dram_tensor("degfT", [128, t_all], F32, kind="ExternalInput")
    degrT_d = nc.dram_tensor("degrT", [128, t_all], F32, kind="ExternalInput")
    dofT_d = nc.dram_tensor("dofT", [128, t_own], F32, kind="ExternalInput")
    dorT_d = nc.dram_tensor("dorT", [128, t_own], F32, kind="ExternalInput")
    ilof_d = nc.dram_tensor("ilof", [128, lo_cols], I16, kind="ExternalInput")
    ihif_d = nc.dram_tensor("ihif", [128, hi_cols], I16, kind="ExternalInput")
    ilor_d = nc.dram_tensor("ilor", [128, lo_cols], I16, kind="ExternalInput")
    ihir_d = nc.dram_tensor("ihir", [128, hi_cols], I16, kind="ExternalInput")
    dstf_d = nc.dram_tensor("dstf", [128, t_own * bt], BF16, kind="ExternalInput")
    dstr_d = nc.dram_tensor("dstr", [128, t_own * bt], BF16, kind="ExternalInput")
    out_d = nc.dram_tensor("out", [sh_pad, 128], F32, kind="ExternalOutput")

    # AllGather chunking of own hT columns (tile granularity)
    chunk_tiles = []
    base = 0
    for j in range(ag_chunks):
        n = (t_own - base) // (ag_chunks - j)
        if n > 0:
            chunk_tiles.append((base, n))
            base += n

    from contextlib import ExitStack

    with tile.TileContext(nc) as tc, ExitStack() as ctx:
        sb = ctx.enter_context(tc.tile_pool(name="sb", bufs=1))
        ps = ctx.enter_context(tc.tile_pool(name="ps", bufs=1, space="PSUM"))
        dr = ctx.enter_context(tc.tile_pool(name="dr", bufs=1, space="DRAM"))

        # ---- DRAM internals: half-tables per (direction, layer) ----
        def half_tables(name):
            return (
                dr.tile([half_t * 128, 128], BF16, name=name + "a"),
                dr.tile([half_t * 128, 128], BF16, name=name + "b"),
            )

        HF1 = half_tables("HF1")
        HR1 = half_tables("HR1")
        HF2 = half_tables("HF2")
        HR2 = half_tables("HR2")
        HTO = [
            dr.tile([128, n * 128], BF16, name=f"HTO{j}")
            for j, (_, n) in enumerate(chunk_tiles)
        ]
        HTF = [
            dr.tile([w, 128, n * 128], BF16, name=f"HTF{j}", addr_space="Shared")
            for j, (_, n) in enumerate(chunk_tiles)
        ]

        # ---- constants / small persistent SBUF ----
        def load_const(dram, shape, dtype=F32, name=None):
            t = sb.tile(shape, dtype, name=name or dram.name + "_sb")
            nc.sync.dma_start(out=t[:], in_=dram[:])
            return t

        def load_cast_bf16(dram, name):
            t32 = sb.tile([128, 128], F32, name=name + "_f32")
            nc.sync.dma_start(out=t32[:], in_=dram[:])
            t16 = sb.tile([128, 128], BF16, name=name)
            nc.vector.tensor_copy(out=t16[:], in_=t32[:])
            return t16

        W1b = load_cast_bf16(W1_d, "W1b")
        W2b = load_cast_bf16(W2_d, "W2b")
        w11Tb = load_cast_bf16(w11T_d, "w11Tb")
        w12Tb = load_cast_bf16(w12T_d, "w12Tb")
        w21Tb = load_cast_bf16(w21T_d, "w21Tb")
        w22Tb = load_cast_bf16(w22T_d, "w22Tb")
        b1c = load_const(b1c_d, [128, 1], name="b1c")
        b2c = load_const(b2c_d, [128, 1], name="b2c")
        bc1r = load_const(bc1r_d, [128, 128], name="bc1r") if add_bc1 else None
        bc2r = load_const(bc2r_d, [128, 128], name="bc2r") if add_bc2 else None

        def load_dinv(dram, cols, name):
            dg = sb.tile([128, cols], F32, name=name + "_deg")
            nc.sync.dma_start(out=dg[:], in_=dram[:])
            sq = sb.tile([128, cols], F32, name=name + "_sq")
            nc.scalar.sqrt(sq[:], dg[:])
            dv = sb.tile([128, cols], F32, name=name)
            nc.vector.reciprocal(out=dv[:], in_=sq[:])
            return dv

        dinvfT = load_dinv(degfT_d, t_all, "dinvfT")
        dinvrT = load_dinv(degrT_d, t_all, "dinvrT")
        dinvofT = load_dinv(dofT_d, t_own, "dinvofT")
        dinvorT = load_dinv(dorT_d, t_own, "dinvorT")

        dstf = load_const(dstf_d, [128, t_own * bt], BF16, "dstf_sb")
        dstr = load_const(dstr_d, [128, t_own * bt], BF16, "dstr_sb")

        iota_i = sb.tile([128, 128], I32, name="iota_i")
        nc.gpsimd.iota(iota_i[:], pattern=[[1, 128]], base=0, channel_multiplier=0)
        iota_bf = sb.tile([128, 128], BF16, name="iota_bf")
        nc.vector.tensor_copy(out=iota_bf[:], in_=iota_i[:])

        ident_bf = sb.tile([128, 128], BF16, name="ident_bf")
        make_identity(nc, ident_bf[:])
        ident_f32 = sb.tile([128, 128], F32, name="ident_f32")
        make_identity(nc, ident_f32[:])

        # -------------------------------------------------------------------
        # dense transform over ALL node tiles
        # -------------------------------------------------------------------
        slab = 1
        for cand in range(min(8, half_t), 0, -1):
            if half_t % cand == 0:
                slab = cand
                break

        def dense_pass(layer, HFt, HRt, Wb, src_slab_loader):
            for s in range(t_all // slab):
                t0 = s * slab
                xs_bf = src_slab_loader(t0, slab)
                hfs = sb.tile([128, slab * 128], BF16, tag="hfs", bufs=2)
                hrs = sb.tile([128, slab * 128], BF16, tag="hrs", bufs=2)
                for t in range(slab):
                    g = t0 + t
                    ph = ps.tile([128, 128], F32, tag="ph", bufs=2)
                    nc.tensor.matmul(
                        out=ph[:],
                        lhsT=xs_bf[:, t * 128 : (t + 1) * 128],
                        rhs=Wb[:],
                        start=True,
                        stop=True,
                    )
                    nc.vector.tensor_scalar(
                        out=hfs[:, t * 128 : (t + 1) * 128],
                        in0=ph[:],
                        scalar1=dinvfT[:, g : g + 1],
                        scalar2=None,
                        op0=mybir.AluOpType.mult,
                    )
                    nc.scalar.activation(
                        out=hrs[:, t * 128 : (t + 1) * 128],
                        in_=ph[:],
                        func=mybir.ActivationFunctionType.Copy,
                        scale=dinvrT[:, g : g + 1],
                    )
                hx = 0 if t0 < half_t else 1
                r0 = (t0 - hx * half_t) * 128
                for tbl, slb in ((HFt[hx], hfs), (HRt[hx], hrs)):
                    nc.sync.dma_start(
                        out=tbl[r0 : r0 + slab * 128, :].rearrange(
                            "(t p) d -> p t d", p=128
                        ),
                        in_=slb[:].rearrange("p (t d) -> p t d", t=slab),
                    )

        def x_slab_loader(t0, nt):
            xs = sb.tile([128, nt * 128], F32, tag="xslab", bufs=2)
            nc.sync.dma_start(out=xs[:], in_=xT_d[:, t0 * 128 : (t0 + nt) * 128])
            xs_bf = sb.tile([128, nt * 128], BF16, tag="xslab_bf", bufs=2)
            nc.scalar.activation(
                out=xs_bf[:], in_=xs[:], func=mybir.ActivationFunctionType.Copy
            )
            return xs_bf

        dense_pass(1, HF1, HR1, W1b, x_slab_loader)

        # -------------------------------------------------------------------
        # aggregation + gating for own tiles, in gather batches of k tiles
        # -------------------------------------------------------------------
        _qrot = [0]

        def gather_batch(g, ilo_sb_src, ihi_sb_src, HT, tagsuf):
            """One batched dma_gather pair for kg tiles -> msg [128, kg*bt, 128]."""
            kg = kg_of[g]
            msg = sb.tile([128, kg * bt, 128], BF16, tag="msg" + tagsuf, bufs=2)
            ilo = sb.tile([128, kg * b_lo * 8], I16, tag="ilo" + tagsuf, bufs=2)
            nc.sync.dma_start(
                out=ilo[:], in_=ilo_sb_src[:, lo_off[g] : lo_off[g + 1]]
            )
            ihi = sb.tile([128, kg * b_hi * 8], I16, tag="ihi" + tagsuf, bufs=2)
            nc.sync.dma_start(
                out=ihi[:], in_=ihi_sb_src[:, hi_off[g] : hi_off[g + 1]]
            )
            q = _qrot[0]
            _qrot[0] = (q + 1) % 4
            nc.gpsimd.dma_gather(
                out_ap=msg[:, : kg * b_lo, :],
                in_ap=HT[0][:],
                idxs_ap=ilo[:],
                num_idxs=kg * b_lo * 128,
                num_idxs_reg=kg * b_lo * 128,
                elem_size=128,
                single_packet=False,
                queue_num=q,
            )
            q = _qrot[0]
            _qrot[0] = (q + 1) % 4
            nc.gpsimd.dma_gather(
                out_ap=msg[:, kg * b_lo :, :],
                in_ap=HT[1][:],
                idxs_ap=ihi[:],
                num_idxs=kg * b_hi * 128,
                num_idxs_reg=kg * b_hi * 128,
                elem_size=128,
                single_packet=False,
                queue_num=q,
            )
            return msg

        def msg_block(msg, kg, ti, b):
            if b < b_lo:
                return msg[:, ti * b_lo + b, :]
            return msg[:, kg * b_lo + ti * b_hi + (b - b_lo), :]

        def agg_tile(t, kg, ti, msg, dst_sb, dinvo, bcr, out_dtype, tagsuf):
            agg = ps.tile([128, 128], F32, tag="agg", bufs=2)
            S = sb.tile([128, bt, 128], BF16, tag="S" + tagsuf, bufs=2)
            nc.vector.tensor_tensor(
                out=S[:],
                in0=iota_bf[:]
                .rearrange("p (o d) -> p o d", o=1)
                .to_broadcast([128, bt, 128]),
                in1=dst_sb[:, t * bt : (t + 1) * bt].to_broadcast([128, bt, 128]),
                op=mybir.AluOpType.is_equal,
            )
            for b in range(bt):
                nc.tensor.matmul(
                    out=agg[:],
                    lhsT=S[:, b, :],
                    rhs=msg_block(msg, kg, ti, b),
                    start=(b == 0),
                    stop=(b == bt - 1),
                )
            s2 = sb.tile([128, 128], F32, tag="s2" + tagsuf, bufs=2)
            nc.vector.tensor_scalar(
                out=s2[:],
                in0=agg[:],
                scalar1=dinvo[:, t : t + 1],
                scalar2=None,
                op0=mybir.AluOpType.mult,
            )
            if bcr is not None:
                s3 = sb.tile([128, 128], F32, tag="s3" + tagsuf, bufs=2)
                nc.vector.tensor_tensor(
                    out=s3[:], in0=s2[:], in1=bcr[:], op=mybir.AluOpType.add
                )
                s2 = s3
            od = sb.tile([128, 128], out_dtype, tag="od" + tagsuf, bufs=2)
            nc.scalar.activation(
                out=od[:], in_=s2[:], func=mybir.ActivationFunctionType.Relu
            )
            return od

        def transpose_to_bf16(src, ident, tagsuf):
            tp = ps.tile([128, 128], src.dtype, tag="tp", bufs=2)
            nc.tensor.transpose(out=tp[:], in_=src[:], identity=ident[:])
            oT = sb.tile([128, 128], BF16, tag="oT" + tagsuf, bufs=2)
            nc.vector.tensor_copy(out=oT[:], in_=tp[:])
            return oT

        def chunk_of(t):
            for j, (c0, n) in enumerate(chunk_tiles):
                if c0 <= t < c0 + n:
                    return j, c0
            raise AssertionError

        # ---- layer 1 ----
        for g in range(nb):
            msgf = gather_batch(g, ilof_d, ihif_d, HF1, "f")
            msgr = gather_batch(g, ilor_d, ihir_d, HR1, "r")
            for ti in range(kg_of[g]):
                t = g * k + ti
                o1 = agg_tile(t, kg_of[g], ti, msgf, dstf, dinvofT, bc1r, BF16, "f")
                o2 = agg_tile(t, kg_of[g], ti, msgr, dstr, dinvorT, bc1r, BF16, "r")
                o1T = transpose_to_bf16(o1, ident_bf, "1")
                o2T = transpose_to_bf16(o2, ident_bf, "2")
                zps = ps.tile([128, 128], F32, tag="z", bufs=2)
                nc.tensor.matmul(out=zps[:], lhsT=w11Tb[:], rhs=o1T[:], start=True, stop=False)
                nc.tensor.matmul(out=zps[:], lhsT=w12Tb[:], rhs=o2T[:], start=False, stop=True)
                GT = sb.tile([128, 128], BF16, tag="GT", bufs=2)
                nc.scalar.activation(
                    out=GT[:],
                    in_=zps[:],
                    func=mybir.ActivationFunctionType.Sigmoid,
                    bias=b1c[:, :1],
                )
                dT = sb.tile([128, 128], BF16, tag="dT", bufs=2)
                nc.vector.tensor_tensor(
                    out=dT[:], in0=o1T[:], in1=o2T[:], op=mybir.AluOpType.subtract
                )
                pT = sb.tile([128, 128], BF16, tag="pT", bufs=2)
                nc.vector.tensor_tensor(
                    out=pT[:], in0=GT[:], in1=dT[:], op=mybir.AluOpType.mult
                )
                hT = sb.tile([128, 128], BF16, tag="hT", bufs=2)
                nc.vector.tensor_tensor(
                    out=hT[:], in0=pT[:], in1=o2T[:], op=mybir.AluOpType.add
                )
                j, c0 = chunk_of(t)
                nc.sync.dma_start(
                    out=HTO[j][:, (t - c0) * 128 : (t - c0 + 1) * 128], in_=hT[:]
                )

        # ---- exchange ----
        for j in range(len(chunk_tiles)):
            nc.gpsimd.collective_compute(
                "AllGather",
                mybir.AluOpType.bypass,
                replica_groups=[list(range(w))],
                ins=[HTO[j].opt()],
                outs=[HTF[j].opt()],
            )

        # ---- layer 2 dense over gathered full hT ----
        def h_slab_loader(t0, nt):
            hs = sb.tile([128, nt * 128], BF16, tag="hslab", bufs=2)
            done = 0
            while done < nt:
                g = t0 + done
                r = g // t_own
                tl = g % t_own
                j, c0 = chunk_of(tl)
                ncy = chunk_tiles[j][1]
                take = min(nt - done, c0 + ncy - tl)
                nc.sync.dma_start(
                    out=hs[:, done * 128 : (done + take) * 128],
                    in_=HTF[j][r, :, (tl - c0) * 128 : (tl - c0 + take) * 128],
                )
                done += take
            return hs

        dense_pass(2, HF2, HR2, W2b, h_slab_loader)

        # ---- layer 2 agg + gate + output ----
        for g in range(nb):
            msgf = gather_batch(g, ilof_d, ihif_d, HF2, "f")
            msgr = gather_batch(g, ilor_d, ihir_d, HR2, "r")
            for ti in range(kg_of[g]):
                t = g * k + ti
                p1 = agg_tile(t, kg_of[g], ti, msgf, dstf, dinvofT, bc2r, F32, "f")
                p2 = agg_tile(t, kg_of[g], ti, msgr, dstr, dinvorT, bc2r, F32, "r")
                p1T = transpose_to_bf16(p1, ident_f32, "1")
                p2T = transpose_to_bf16(p2, ident_f32, "2")
                zps = ps.tile([128, 128], F32, tag="z", bufs=2)
                nc.tensor.matmul(out=zps[:], lhsT=w21Tb[:], rhs=p1T[:], start=True, stop=False)
                nc.tensor.matmul(out=zps[:], lhsT=w22Tb[:], rhs=p2T[:], start=False, stop=True)
                G2T = sb.tile([128, 128], BF16, tag="GT", bufs=2)
                nc.scalar.activation(
                    out=G2T[:],
                    in_=zps[:],
                    func=mybir.ActivationFunctionType.Sigmoid,
                    bias=b2c[:, :1],
                )
                g2p = ps.tile([128, 128], BF16, tag="tp", bufs=2)
                nc.tensor.transpose(out=g2p[:], in_=G2T[:], identity=ident_bf[:])
                g2s = sb.tile([128, 128], F32, tag="g2s", bufs=2)
                nc.vector.tensor_copy(out=g2s[:], in_=g2p[:])
                dd = sb.tile([128, 128], F32, tag="dd", bufs=2)
                nc.vector.tensor_tensor(
                    out=dd[:], in0=p1[:], in1=p2[:], op=mybir.AluOpType.subtract
                )
                pr = sb.tile([128, 128], F32, tag="pr", bufs=2)
                nc.vector.tensor_tensor(
                    out=pr[:], in0=dd[:], in1=g2s[:], op=mybir.AluOpType.mult
                )
                ot = sb.tile([128, 128], F32, tag="ot", bufs=2)
                nc.vector.tensor_tensor(
                    out=ot[:], in0=pr[:], in1=p2[:], op=mybir.AluOpType.add
                )
                nc.sync.dma_start(out=out_d[t * 128 : (t + 1) * 128, :], in_=ot[:])

    nc.compile()
    return nc


# ---------------------------------------------------------------------------
# full pipeline
# ---------------------------------------------------------------------------


def make_in_maps(inputs, meta, tables):
    w = W_CORES
    sh_real, sh_pad = meta["sh_real"], meta["sh_pad"]
    t_own, t_all, np_pad = meta["t_own"], meta["t_all"], meta["np_pad"]
    n_real = w * sh_real

    x = np.asarray(inputs["x"], np.float32)
    slots = _slot_of(np.arange(n_real), sh_real, sh_pad)
    x_slot = np.zeros((np_pad, D), np.float32)
    x_slot[slots] = x
    xT = np.ascontiguousarray(x_slot.T)

    def t2(a):
        return np.ascontiguousarray(np.asarray(a, np.float32).T)

    W1 = np.asarray(inputs["W1"], np.float32)
    W2 = np.asarray(inputs["W2"], np.float32)
    b1c = np.asarray(inputs["b1"], np.float32).reshape(128, 1)
    b2c = np.asarray(inputs["b2"], np.float32).reshape(128, 1)
    bc1r = np.broadcast_to(np.asarray(inputs["bc1"], np.float32), (128, 128)).copy()
    bc2r = np.broadcast_to(np.asarray(inputs["bc2"], np.float32), (128, 128)).copy()

    degfT = np.ascontiguousarray(tables["deg_f"].reshape(t_all, 128).T)
    degrT = np.ascontiguousarray(tables["deg_r"].reshape(t_all, 128).T)

    in_maps = []
    for c in range(w):
        own0 = c * t_own
        in_maps.append(
            dict(
                xT=xT, W1=W1, W2=W2,
                w11T=t2(inputs["w11"]), w12T=t2(inputs["w12"]),
                w21T=t2(inputs["w21"]), w22T=t2(inputs["w22"]),
                b1c=b1c, b2c=b2c, bc1r=bc1r, bc2r=bc2r,
                degfT=degfT, degrT=degrT,
                dofT=np.ascontiguousarray(degfT[:, own0 : own0 + t_own]),
                dorT=np.ascontiguousarray(degrT[:, own0 : own0 + t_own]),
                ilof=tables["ilo_f"][c], ihif=tables["ihi_f"][c],
                ilor=tables["ilo_r"][c], ihir=tables["ihi_r"][c],
                dstf=tables["dst_f"][c], dstr=tables["dst_r"][c],
            )
        )
    return in_maps


def assemble_output(results, meta):
    sh_real, sh_pad = meta["sh_real"], meta["sh_pad"]
    n_real = W_CORES * sh_real
    full = np.concatenate([r["out"] for r in results], axis=0)
    slots = _slot_of(np.arange(n_real), sh_real, sh_pad)
    return np.ascontiguousarray(full[slots]).astype(np.float32)


_CACHE = {}


def _get_program(meta, add_bc1, add_bc2):
    key = (meta["t_own"], meta["b_lo"], meta["b_hi"], meta["k"], add_bc1, add_bc2)
    if key not in _CACHE:
        _CACHE[key] = build_program(
            meta["t_own"], meta["b_lo"], meta["b_hi"], meta["k"],
            add_bc1=add_bc1, add_bc2=add_bc2,
        )
    return _CACHE[key]


def _install_ntff_hook():
    """Shim antenv.axon_hooks (absent in this image) so run_bass_kernel_spmd
    trace=True can capture NTFF profiles via libaxon_pjrt.so ctypes calls."""
    import contextlib
    import ctypes
    import types

    if "antenv.axon_hooks" in sys.modules:
        return
    so_path = "/opt/axon/libaxon_pjrt.so"
    holder = {}
    m = types.ModuleType("antenv.axon_hooks")
    m.set_axon_ntff_profile_hook = lambda h: holder.__setitem__("h", h)
    m.get_axon_ntff_profile_hook = lambda: holder.get("h")
    sys.modules["antenv.axon_hooks"] = m
    try:
        import antenv

        antenv.axon_hooks = m
    except ImportError:
        pass
    try:
        lib = ctypes.CDLL(so_path)
    except OSError:
        return
    if not hasattr(lib, "axon_start_nrt_profile"):
        return
    lib.axon_start_nrt_profile.argtypes = [
        ctypes.POINTER(ctypes.c_int64),
        ctypes.c_size_t,
    ]
    lib.axon_start_nrt_profile.restype = ctypes.c_int64
    lib.axon_stop_nrt_profile.argtypes = [ctypes.c_char_p]
    lib.axon_stop_nrt_profile.restype = ctypes.c_int64

    @contextlib.contextmanager
    def _hook(output_dir, device_ids):
        import jax

        jax.devices()
        if device_ids:
            ids = (ctypes.c_int64 * len(device_ids))(*device_ids)
            rc = lib.axon_start_nrt_profile(ids, len(device_ids))
        else:
            rc = lib.axon_start_nrt_profile(None, 0)
        if rc != 0:
            raise RuntimeError(f"axon_start_nrt_profile rc={rc}")
        try:
            yield
        finally:
            n = lib.axon_stop_nrt_profile(str(output_dir).encode())
            print(f"profile: {n} file(s) written to {output_dir}", file=sys.stderr)

    holder["h"] = _hook


def _patch_upload_artifacts():
    import concourse.bass_utils as bu

    bu.upload_artifacts = lambda tmpdir: tmpdir


def kernel(**inputs):
    x = np.asarray(inputs["x"], np.float32)
    n_real = x.shape[0]
    meta, tables = host_prepare(x, np.asarray(inputs["edge_index"]), n_real)
    add_bc1 = bool(np.any(np.asarray(inputs["bc1"]) != 0))
    add_bc2 = bool(np.any(np.asarray(inputs["bc2"]) != 0))
    nc = _get_program(meta, add_bc1, add_bc2)
    in_maps = make_in_maps(inputs, meta, tables)
    if bool(int(os.environ.get("KERNEL_TRACE", "0"))):
        _install_ntff_hook()
        _patch_upload_artifacts()
    res = run_bass_kernel_spmd(
        nc,
        in_maps,
        core_ids=list(range(W_CORES)),
        trace=bool(int(os.environ.get("KERNEL_TRACE", "0"))),
    )
    global LAST_EXEC_NS
    LAST_EXEC_NS = res.exec_time_ns
    if res.exec_time_ns is not None:
        print(f"HW exec time: {res.exec_time_ns} ns")
    return assemble_output(res.results, meta)


LAST_EXEC_NS = None

